# revision 18
# baseline (speedup 1.0000x reference)
"""Trainium2 Bass kernel for nn_EntropyLoss_84542136254557.

Computes: transform src by (R, t), pairwise sq-distances to tgt [B, N, N],
min over tgt -> nearest-neighbor distance per src point, stable top-k=512
selection, gather log(sampling_scores[b, j, idx_k[b, j]]), mean loss.

Device does the O(N^2) distance+min field (the dominant compute):
  d[n, m] = xx[n] - 2*<src_corr[:,n], tgt[:,m]> + yy[m]
          = <a_n, b_m> with a_n = [-2*sc, xx_n, 1] (5 terms)
                            b_m = [t, 1, yy_m]
To run the PE at full rate with near-fp32 accuracy, each fp32 operand is
split into fp16 hi+lo halves and the four cross products are folded into a
single K=20 contraction (contraction depth <= 32 is free on the 128x128 PE):
  d ~= [a_hi; a_lo; a_hi; a_lo]^T . [b_hi; b_hi; b_lo; b_lo]
One K=20 fp16 matmul per [128 src x 512 tgt] tile -> PSUM (fp32 accum).

The min-reduction over targets is split across engines: even chunks stay in
PSUM, odd chunks are copied PSUM->SBUF by ScalarE, and a custom DVE op
(body=min(Src0,Src1), accum=min) consumes one PSUM and one SBUF chunk at
2 elements/cycle on VectorE. TensorE / VectorE / ScalarE end up balanced at
~158us each per core (~182us span).

Sharding: 8 cores = 4 batches x 2 halves of the 8192 src points. Each core:
32 row-tiles (128 src each) x 16 col-chunks. Row-tiles are packed into PE
row-groups (partition offsets 0/32/64/96) so matmuls from different groups
execute concurrently.

Device nearest-distances are accurate to ~1e-4; the true top-512 is
recovered exactly on the host by re-evaluating the best 768 rows per batch
in the reference's fp32 op order (verified bitwise-equal to XLA-CPU) and
ranking those. The final gather/log/mean is a tiny [4, 512] host op.
"""

import numpy as np

import concourse.bacc as bacc
import concourse.mybir as mybir
import concourse.dve_ops as _dve_ops
from concourse.dve_ops import DveOp
from concourse.dve_spec import Spec, Src0, Src1, C0, minn, lower as _dve_lower
from concourse.dve_uop import DveOpSpec
from concourse.tile import TileContext
from concourse.bass_utils import run_bass_kernel_spmd

_TTMIN_NAME = "TENSOR_TENSOR_MIN_REDUCE_ANT"


def _ttmin_ref(in0, in1, c0, c1, c2):
    return np.minimum(in0.astype(np.float32), in1.astype(np.float32))


def _get_ttmin_op():
    """Custom DVE op: out = min(in0, in1), accum_out = min-reduce(out, init=s0).

    Consumes two tensors per cycle on VectorE (both read ports), halving the
    min-reduction time vs a plain tensor_reduce."""
    if _TTMIN_NAME in _dve_ops._SUB_OPCODE_FOR_NAME:
        for op in _dve_ops.OPS:
            if op.name == _TTMIN_NAME:
                return op
    spec = Spec(body=minn(Src0, Src1), accum=minn, accum_init=C0,
                reference=_ttmin_ref)
    row = _dve_ops._CUSTOM_DVE_ROW_BASE + len(_dve_ops.OPS)
    assert row < 0x20
    uops = _dve_lower(spec, ver="v3")
    sha = DveOpSpec(name=_TTMIN_NAME, opcode=row, uops=uops, rd1_en=True).sha("v3")
    op = DveOp(_TTMIN_NAME, spec, subdim=False, uops_sha={"v3": sha})
    _dve_ops.OPS.append(op)
    _dve_ops._SUB_OPCODE_FOR_NAME[_TTMIN_NAME] = row
    _dve_ops.CUSTOM_DVE_SPECS[_TTMIN_NAME] = spec
    return op

B, K, N = 4, 512, 8192
N_CORES = 8
HALF = N // 2            # src rows per core
RT = HALF // 128         # 32 row-tiles per core
KC = 20                  # folded contraction depth (4x 5-term fp16 pieces)
F32 = mybir.dt.float32
F16 = mybir.dt.float16

_nc_cache = {}
last_perf = None         # BassKernelResults of the most recent run (for test.py)


def _build_nc():
    nc = bacc.Bacc("TRN2", target_bir_lowering=False)
    a_ext = nc.declare_dram_parameter("a", [128, (RT // 4) * 128], F16, isOutput=False)
    b_ext = nc.declare_dram_parameter("b", [KC, N], F16, isOutput=False)
    o_ext = nc.declare_dram_parameter("o", [128, RT], F32, isOutput=True)

    ttmin = _get_ttmin_op()
    with TileContext(nc) as tc:
        with (
            tc.tile_pool(name="sb", bufs=1) as sb,
            tc.tile_pool(name="pse", bufs=4, space="PSUM") as ppe,
            tc.tile_pool(name="pso", bufs=4, space="PSUM") as ppo,
            tc.tile_pool(name="cp", bufs=6) as cpp,
        ):
            a_sb = sb.tile([128, (RT // 4) * 128], F16)
            b_sb = sb.tile([128, N], F16)
            # Split input DMAs to chunk-pair granularity so the first
            # matmuls start after ~40KB instead of the full target set.
            nc.sync.dma_start(out=a_sb[:, 0:128], in_=a_ext[:, 0:128])
            for p in range(8):
                for m in range(4):
                    nc.sync.dma_start(
                        out=b_sb[32 * m : 32 * m + KC, p * 1024 : (p + 1) * 1024],
                        in_=b_ext[:, p * 1024 : (p + 1) * 1024],
                    )
                if p == 0:
                    nc.sync.dma_start(
                        out=a_sb[:, 128 : (RT // 4) * 128],
                        in_=a_ext[:, 128 : (RT // 4) * 128],
                    )

            acc = sb.tile([128, RT * 8], F32)
            out_sb = sb.tile([128, RT], F32)

            def mk_mm(j, chunk, ps, half):
                m = j % 4
                q = j // 4
                nc.tensor.matmul(
                    out=ps[:, half * 512 : (half + 1) * 512],
                    lhsT=a_sb[32 * m : 32 * m + KC, q * 128 : (q + 1) * 128],
                    rhs=b_sb[32 * m : 32 * m + KC, chunk * 512 : (chunk + 1) * 512],
                    start=True,
                    stop=True,
                    tile_position=(32 * m, 0),
                )

            # Row-tiles processed in quads across the 4 PE row-groups (their
            # matmuls stream concurrently on disjoint PE rows). Per chunk-pair
            # p: the even chunk [128,512] stays in PSUM (in0), the odd chunk
            # is copied PSUM->SBUF by ScalarE (in1); the custom min-min DVE op
            # consumes both at 2 elements/cycle and emits the running min.
            for jq in range(RT // 4):
                for p in range(8):
                    pes = [
                        ppe.tile([128, 512], F32, tag="pse", name=f"pe{jq}_{p}_{i}")
                        for i in range(4)
                    ]
                    pos = [
                        ppo.tile([128, 512], F32, tag="pso", name=f"po{jq}_{p}_{i}")
                        for i in range(4)
                    ]
                    for m in range(4):
                        mk_mm(4 * jq + m, 2 * p, pes[m], 0)
                    for m in range(4):
                        mk_mm(4 * jq + m, 2 * p + 1, pos[m], 0)
                    for m in range(4):
                        j = 4 * jq + m
                        so = cpp.tile([128, 512], F32, tag="so", name=f"so_{jq}_{p}_{m}")
                        sc = cpp.tile([128, 512], F32, tag="sc", name=f"sc_{jq}_{p}_{m}")
                        nc.scalar.copy(out=so[:, :], in_=pos[m][:, :])
                        nc.vector._custom_dve(
                            ttmin,
                            out=sc[:, :],
                            in0=pes[m][:, :],
                            in1=so[:, :],
                            s0=3.0e38,
                            accum_out=acc[:, j * 8 + p : j * 8 + p + 1],
                        )
                # fold this quad's partial minima and ship them while later
                # quads are still computing -- keeps the kernel tail to one
                # small reduce + 2KB DMA.
                nc.vector.tensor_reduce(
                    out=out_sb[:, 4 * jq : 4 * jq + 4],
                    in_=acc[:, 32 * jq : 32 * jq + 32].rearrange(
                        "p (j pp) -> p j pp", pp=8
                    ),
                    axis=mybir.AxisListType.X,
                    op=mybir.AluOpType.min,
                )
                nc.sync.dma_start(
                    out=o_ext[:, 4 * jq : 4 * jq + 4],
                    in_=out_sb[:, 4 * jq : 4 * jq + 4],
                )

    nc.finalize()
    return nc


def _get_nc():
    if "nc" not in _nc_cache:
        _nc_cache["nc"] = _build_nc()
    return _nc_cache["nc"]


def _split16(x):
    hi = x.astype(np.float16)
    lo = (x - hi.astype(np.float32)).astype(np.float16)
    return hi, lo


def _pack_a(a_core):
    """a_core [5, HALF] fp32 -> [128, (RT//4)*128] fp16; row-tile j sits at
    partition 32*(j%4), columns (j//4)*128:..., as [a_hi; a_lo; a_hi; a_lo]."""
    hi, lo = _split16(a_core)
    stacked = np.concatenate([hi, lo, hi, lo], axis=0)  # [20, HALF]
    out = np.zeros((128, (RT // 4) * 128), dtype=np.float16)
    for j in range(RT):
        m = j % 4
        q = j // 4
        out[32 * m : 32 * m + KC, q * 128 : (q + 1) * 128] = stacked[
            :, j * 128 : (j + 1) * 128
        ]
    return out


def kernel(sampling_scores, src, tgt, rotation_ab, translation_ab, _trace=False):
    global last_perf
    sampling_scores = np.asarray(sampling_scores, dtype=np.float32)
    src = np.asarray(src, dtype=np.float32)
    tgt = np.asarray(tgt, dtype=np.float32)
    rotation_ab = np.asarray(rotation_ab, dtype=np.float32)
    translation_ab = np.asarray(translation_ab, dtype=np.float32)

    # src_corr = R @ src + t  (fp32, tiny)
    src_corr = np.matmul(rotation_ab, src) + translation_ab[:, :, None]
    xx = np.sum(src_corr * src_corr, axis=1)  # [B, N]
    yy = np.sum(tgt * tgt, axis=1)            # [B, N]

    ones = np.ones((B, 1, N), dtype=np.float32)
    a_full = np.concatenate([-2.0 * src_corr, xx[:, None, :], ones], axis=1)  # [B,5,N]
    b_full = np.concatenate([tgt, ones, yy[:, None, :]], axis=1)              # [B,5,N]

    in_maps = []
    b_packed = []
    for b_idx in range(B):
        bhi, blo = _split16(b_full[b_idx])
        b_packed.append(
            np.ascontiguousarray(np.concatenate([bhi, bhi, blo, blo], axis=0))
        )
    for c in range(N_CORES):
        b_idx, h = divmod(c, 2)
        a_core = a_full[b_idx, :, h * HALF : (h + 1) * HALF]
        in_maps.append({"a": _pack_a(a_core), "b": b_packed[b_idx]})

    nc = _get_nc()
    res = run_bass_kernel_spmd(
        nc, in_maps, core_ids=list(range(N_CORES)), trace=_trace
    )
    last_perf = res

    nearst = np.empty((B, N), dtype=np.float32)
    for c in range(N_CORES):
        b_idx, h = divmod(c, 2)
        o = res.results[c]["o"]  # [128, RT]; o[p, j] = row j*128+p
        nearst[b_idx, h * HALF : (h + 1) * HALF] = o.T.reshape(-1)

    global _last_nearst
    _last_nearst = nearst

    # The device nearst differs from a strict-fp32 CPU evaluation by up to
    # ~1e-4 (fp16-split matmul), enough to swap near-tied ranks. Re-evaluate
    # the best NCAND rows per batch exactly in the reference's fp32 op order
    # (verified bitwise-equal to XLA-CPU), then rank those.
    NCAND = 768  # reference gap between rank 512 and 768 is ~2.5e-3 >> 1e-4
    idx_k = np.empty((B, K), dtype=np.int64)
    for b_idx in range(B):
        cand = np.sort(np.argpartition(nearst[b_idx], NCAND)[:NCAND])
        sc = src_corr[b_idx][:, cand]                      # [3, NCAND]
        inner = -2.0 * np.matmul(sc.T, tgt[b_idx])         # [NCAND, N] fp32
        d = (xx[b_idx][cand][:, None] + inner) + yy[b_idx][None, :]
        exact = d.min(axis=1)                              # [NCAND] fp32
        order = np.argsort(exact, kind="stable")[:K]       # stable => index tiebreak
        idx_k[b_idx] = cand[order]

    j_idx = np.arange(K)
    sel = sampling_scores[np.arange(B)[:, None], j_idx[None, :], idx_k]  # [B, K]
    loss = -np.log(sel.astype(np.float64)).sum(axis=1) / float(K)
    return np.float32(loss.mean())


# revision 19
# speedup vs baseline: 1.0079x; 1.0079x over previous
"""Trainium2 Bass kernel for nn_EntropyLoss_84542136254557.

Computes: transform src by (R, t), pairwise sq-distances to tgt [B, N, N],
min over tgt -> nearest-neighbor distance per src point, stable top-k=512
selection, gather log(sampling_scores[b, j, idx_k[b, j]]), mean loss.

Device does the O(N^2) distance+min field (the dominant compute):
  d[n, m] = xx[n] - 2*<src_corr[:,n], tgt[:,m]> + yy[m]
          = <a_n, b_m> with a_n = [-2*sc, xx_n, 1] (5 terms)
                            b_m = [t, 1, yy_m]
To run the PE at full rate with near-fp32 accuracy, each fp32 operand is
split into fp16 hi+lo halves and the four cross products are folded into a
single K=20 contraction (contraction depth <= 32 is free on the 128x128 PE):
  d ~= [a_hi; a_lo; a_hi; a_lo]^T . [b_hi; b_hi; b_lo; b_lo]
One K=20 fp16 matmul per [128 src x 512 tgt] tile -> PSUM (fp32 accum).

The min-reduction over targets is split across engines: even chunks stay in
PSUM, odd chunks are copied PSUM->SBUF by ScalarE, and a custom DVE op
(body=min(Src0,Src1), accum=min) consumes one PSUM and one SBUF chunk at
2 elements/cycle on VectorE. TensorE / VectorE / ScalarE end up balanced at
~158us each per core (~182us span).

Sharding: 8 cores = 4 batches x 2 halves of the 8192 src points. Each core:
32 row-tiles (128 src each) x 16 col-chunks. Row-tiles are packed into PE
row-groups (partition offsets 0/32/64/96) so matmuls from different groups
execute concurrently.

Device nearest-distances are accurate to ~1e-4; the true top-512 is
recovered exactly on the host by re-evaluating the best 768 rows per batch
in the reference's fp32 op order (verified bitwise-equal to XLA-CPU) and
ranking those. The final gather/log/mean is a tiny [4, 512] host op.
"""

import numpy as np

import concourse.bacc as bacc
import concourse.mybir as mybir
import concourse.dve_ops as _dve_ops
from concourse.dve_ops import DveOp
from concourse.dve_spec import Spec, Src0, Src1, C0, minn, lower as _dve_lower
from concourse.dve_uop import DveOpSpec
from concourse.tile import TileContext
from concourse.bass_utils import run_bass_kernel_spmd

_TTMIN_NAME = "TENSOR_TENSOR_MIN_REDUCE_ANT"


def _ttmin_ref(in0, in1, c0, c1, c2):
    return np.minimum(in0.astype(np.float32), in1.astype(np.float32))


def _get_ttmin_op():
    """Custom DVE op: out = min(in0, in1), accum_out = min-reduce(out, init=s0).

    Consumes two tensors per cycle on VectorE (both read ports), halving the
    min-reduction time vs a plain tensor_reduce."""
    if _TTMIN_NAME in _dve_ops._SUB_OPCODE_FOR_NAME:
        for op in _dve_ops.OPS:
            if op.name == _TTMIN_NAME:
                return op
    spec = Spec(body=minn(Src0, Src1), accum=minn, accum_init=C0,
                reference=_ttmin_ref)
    row = _dve_ops._CUSTOM_DVE_ROW_BASE + len(_dve_ops.OPS)
    assert row < 0x20
    uops = _dve_lower(spec, ver="v3")
    sha = DveOpSpec(name=_TTMIN_NAME, opcode=row, uops=uops, rd1_en=True).sha("v3")
    op = DveOp(_TTMIN_NAME, spec, subdim=False, uops_sha={"v3": sha})
    _dve_ops.OPS.append(op)
    _dve_ops._SUB_OPCODE_FOR_NAME[_TTMIN_NAME] = row
    _dve_ops.CUSTOM_DVE_SPECS[_TTMIN_NAME] = spec
    return op

B, K, N = 4, 512, 8192
N_CORES = 8
HALF = N // 2            # src rows per core
RT = HALF // 128         # 32 row-tiles per core
KC = 20                  # folded contraction depth (4x 5-term fp16 pieces)
F32 = mybir.dt.float32
F16 = mybir.dt.float16

_nc_cache = {}
last_perf = None         # BassKernelResults of the most recent run (for test.py)


def _build_nc():
    nc = bacc.Bacc("TRN2", target_bir_lowering=False)
    a_ext = nc.declare_dram_parameter("a", [128, (RT // 4) * 128], F16, isOutput=False)
    b_ext = nc.declare_dram_parameter("b", [KC, N], F16, isOutput=False)
    o_ext = nc.declare_dram_parameter("o", [128, RT], F32, isOutput=True)

    ttmin = _get_ttmin_op()
    with TileContext(nc) as tc:
        with (
            tc.tile_pool(name="sb", bufs=1) as sb,
            tc.tile_pool(name="pse", bufs=4, space="PSUM") as ppe,
            tc.tile_pool(name="pso", bufs=4, space="PSUM") as ppo,
            tc.tile_pool(name="cp", bufs=8) as cpp,
        ):
            a_sb = sb.tile([128, (RT // 4) * 128], F16)
            b_sb = sb.tile([128, N], F16)
            # Split input DMAs so the first quad's matmuls can start before
            # the whole weight/target set has landed.
            nc.sync.dma_start(out=a_sb[:, 0:128], in_=a_ext[:, 0:128])
            for m in range(4):
                nc.sync.dma_start(
                    out=b_sb[32 * m : 32 * m + KC, 0 : N // 2],
                    in_=b_ext[:, 0 : N // 2],
                )
            nc.sync.dma_start(
                out=a_sb[:, 128 : (RT // 4) * 128], in_=a_ext[:, 128 : (RT // 4) * 128]
            )
            for m in range(4):
                nc.sync.dma_start(
                    out=b_sb[32 * m : 32 * m + KC, N // 2 : N],
                    in_=b_ext[:, N // 2 : N],
                )

            acc = sb.tile([128, RT * 8], F32)
            out_sb = sb.tile([128, RT], F32)

            def mk_mm(j, chunk, ps, half):
                m = j % 4
                q = j // 4
                nc.tensor.matmul(
                    out=ps[:, half * 512 : (half + 1) * 512],
                    lhsT=a_sb[32 * m : 32 * m + KC, q * 128 : (q + 1) * 128],
                    rhs=b_sb[32 * m : 32 * m + KC, chunk * 512 : (chunk + 1) * 512],
                    start=True,
                    stop=True,
                    tile_position=(32 * m, 0),
                )

            # Row-tiles processed in quads across the 4 PE row-groups (their
            # matmuls stream concurrently on disjoint PE rows). Per chunk-pair
            # p: the even chunk [128,512] stays in PSUM (in0), the odd chunk
            # is copied PSUM->SBUF by ScalarE (in1); the custom min-min DVE op
            # consumes both at 2 elements/cycle and emits the running min.
            for jq in range(RT // 4):
                for p in range(8):
                    pes = [
                        ppe.tile([128, 512], F32, tag="pse", name=f"pe{jq}_{p}_{i}")
                        for i in range(4)
                    ]
                    pos = [
                        ppo.tile([128, 512], F32, tag="pso", name=f"po{jq}_{p}_{i}")
                        for i in range(4)
                    ]
                    for m in range(4):
                        mk_mm(4 * jq + m, 2 * p, pes[m], 0)
                    for m in range(4):
                        mk_mm(4 * jq + m, 2 * p + 1, pos[m], 0)
                    for m in range(4):
                        j = 4 * jq + m
                        so = cpp.tile([128, 512], F32, tag="so", name=f"so_{jq}_{p}_{m}")
                        dmy = cpp.tile([128, 1], F32, tag="dmy", name=f"dmy_{jq}_{p}_{m}")
                        nc.scalar.copy(out=so[:, :], in_=pos[m][:, :])
                        nc.vector._custom_dve(
                            ttmin,
                            out=dmy.broadcast_to((128, 512)),
                            in0=pes[m][:, :],
                            in1=so[:, :],
                            s0=3.0e38,
                            accum_out=acc[:, j * 8 + p : j * 8 + p + 1],
                        )
                # fold this quad's partial minima and ship them while later
                # quads are still computing -- keeps the kernel tail to one
                # small reduce + 2KB DMA.
                nc.vector.tensor_reduce(
                    out=out_sb[:, 4 * jq : 4 * jq + 4],
                    in_=acc[:, 32 * jq : 32 * jq + 32].rearrange(
                        "p (j pp) -> p j pp", pp=8
                    ),
                    axis=mybir.AxisListType.X,
                    op=mybir.AluOpType.min,
                )
                nc.sync.dma_start(
                    out=o_ext[:, 4 * jq : 4 * jq + 4],
                    in_=out_sb[:, 4 * jq : 4 * jq + 4],
                )

    nc.finalize()
    return nc


def _get_nc():
    if "nc" not in _nc_cache:
        _nc_cache["nc"] = _build_nc()
    return _nc_cache["nc"]


def _split16(x):
    hi = x.astype(np.float16)
    lo = (x - hi.astype(np.float32)).astype(np.float16)
    return hi, lo


def _pack_a(a_core):
    """a_core [5, HALF] fp32 -> [128, (RT//4)*128] fp16; row-tile j sits at
    partition 32*(j%4), columns (j//4)*128:..., as [a_hi; a_lo; a_hi; a_lo]."""
    hi, lo = _split16(a_core)
    stacked = np.concatenate([hi, lo, hi, lo], axis=0)  # [20, HALF]
    out = np.zeros((128, (RT // 4) * 128), dtype=np.float16)
    for j in range(RT):
        m = j % 4
        q = j // 4
        out[32 * m : 32 * m + KC, q * 128 : (q + 1) * 128] = stacked[
            :, j * 128 : (j + 1) * 128
        ]
    return out


def kernel(sampling_scores, src, tgt, rotation_ab, translation_ab, _trace=False):
    global last_perf
    sampling_scores = np.asarray(sampling_scores, dtype=np.float32)
    src = np.asarray(src, dtype=np.float32)
    tgt = np.asarray(tgt, dtype=np.float32)
    rotation_ab = np.asarray(rotation_ab, dtype=np.float32)
    translation_ab = np.asarray(translation_ab, dtype=np.float32)

    # src_corr = R @ src + t  (fp32, tiny)
    src_corr = np.matmul(rotation_ab, src) + translation_ab[:, :, None]
    xx = np.sum(src_corr * src_corr, axis=1)  # [B, N]
    yy = np.sum(tgt * tgt, axis=1)            # [B, N]

    ones = np.ones((B, 1, N), dtype=np.float32)
    a_full = np.concatenate([-2.0 * src_corr, xx[:, None, :], ones], axis=1)  # [B,5,N]
    b_full = np.concatenate([tgt, ones, yy[:, None, :]], axis=1)              # [B,5,N]

    in_maps = []
    b_packed = []
    for b_idx in range(B):
        bhi, blo = _split16(b_full[b_idx])
        b_packed.append(
            np.ascontiguousarray(np.concatenate([bhi, bhi, blo, blo], axis=0))
        )
    for c in range(N_CORES):
        b_idx, h = divmod(c, 2)
        a_core = a_full[b_idx, :, h * HALF : (h + 1) * HALF]
        in_maps.append({"a": _pack_a(a_core), "b": b_packed[b_idx]})

    nc = _get_nc()
    res = run_bass_kernel_spmd(
        nc, in_maps, core_ids=list(range(N_CORES)), trace=_trace
    )
    last_perf = res

    nearst = np.empty((B, N), dtype=np.float32)
    for c in range(N_CORES):
        b_idx, h = divmod(c, 2)
        o = res.results[c]["o"]  # [128, RT]; o[p, j] = row j*128+p
        nearst[b_idx, h * HALF : (h + 1) * HALF] = o.T.reshape(-1)

    global _last_nearst
    _last_nearst = nearst

    # The device nearst differs from a strict-fp32 CPU evaluation by up to
    # ~1e-4 (fp16-split matmul), enough to swap near-tied ranks. Re-evaluate
    # the best NCAND rows per batch exactly in the reference's fp32 op order
    # (verified bitwise-equal to XLA-CPU), then rank those.
    NCAND = 768  # reference gap between rank 512 and 768 is ~2.5e-3 >> 1e-4
    idx_k = np.empty((B, K), dtype=np.int64)
    for b_idx in range(B):
        cand = np.sort(np.argpartition(nearst[b_idx], NCAND)[:NCAND])
        sc = src_corr[b_idx][:, cand]                      # [3, NCAND]
        inner = -2.0 * np.matmul(sc.T, tgt[b_idx])         # [NCAND, N] fp32
        d = (xx[b_idx][cand][:, None] + inner) + yy[b_idx][None, :]
        exact = d.min(axis=1)                              # [NCAND] fp32
        order = np.argsort(exact, kind="stable")[:K]       # stable => index tiebreak
        idx_k[b_idx] = cand[order]

    j_idx = np.arange(K)
    sel = sampling_scores[np.arange(B)[:, None], j_idx[None, :], idx_k]  # [B, K]
    loss = -np.log(sel.astype(np.float64)).sum(axis=1) / float(K)
    return np.float32(loss.mean())
